# revision 53
# baseline (speedup 1.0000x reference)
"""Trainium2 Bass kernel for multi-head self-attention with RoPE.

Problem: x[2,2048,2048] f32, Wq/Wk/Wv/Wo [2048,2048], causal MHA, 16 heads,
dk=128, RoPE on Q/K. Sharding: 8 cores = 2 batches x 4 head-groups
(4 heads/core). Each core computes its batch's partial output projection for
its 4 heads; host sums the 4 partials per batch.

Device-side scheme (per core, all matmuls bf16 with f32 PSUM accumulation):
  - host pre-transposes x -> xT [D,S] and weight slices; RoPE pair
    de-interleave is folded into a row permutation of Wq/Wk so the rotation
    becomes partition-block ops; 1/sqrt(dk) folded into Wq/Wk.
  - QT/KT [dk,S] = W-slice^T-tiles @ xT-tiles (+RoPE, bf16 vector ops)
  - V [S,dk] with a ones column appended (interleaved [.,516] layout)
  - ST tiles [k,q] = KT-slice^T @ QT; exp on ScalarE; causal mask only on
    diagonal tiles (multiply by precomputed triangular mask)
  - ctx [q, dk+1] = expST^T @ V_aug; column dk holds the softmax denominator
  - ctx scaled by 1/denom during PSUM eviction, transposed via PE to ctxT
  - out [S, E] = ctxT^T @ WoT-slice, accumulated over the 4 head-chunks
Phases are emitted interleaved (projection passes between attention blocks)
so projection matmuls fill PE gaps while ScalarE computes exp.
"""
from contextlib import ExitStack

import numpy as np
import ml_dtypes

try:
    import concourse.bass as bass  # noqa: F401
except ImportError:  # fresh grading dir: repo lives at /opt/trn_rl_repo
    import sys
    sys.path.insert(0, "/opt/trn_rl_repo")

import concourse.bass as bass
import concourse.bass_isa as bass_isa
import concourse.mybir as mybir
import concourse.tile as tile
from concourse import bacc, bass_utils

BF16 = mybir.dt.bfloat16
F32 = mybir.dt.float32
FP8 = mybir.dt.float8e4
PM_DR = mybir.MatmulPerfMode.DoubleRow
AF = mybir.ActivationFunctionType

D = 2048          # model dim
S = 2048          # sequence length
DK = 128          # head dim
HPC = 4           # heads per core
C = HPC * DK      # per-core feature slice = 512
THETA = 10000.0
NCORES = 8

_NC = None  # cached compiled Bass module


def _build_program(repeat=1):
    nc = bacc.Bacc("TRN2", debug=False, num_devices=NCORES)

    xT_d = nc.dram_tensor("xT", [D, S], BF16, kind="ExternalInput")
    # fp8 copies for the Q/K projections (DoubleRow): x8[t, p, s] = xT[t*128+p, s];
    # wq8/wk8[h, p, g, j, c] = (WqT_perm * 2048)[(2g+j)*128+p, h*DK+c]
    x8_d = nc.dram_tensor("x8", [D // 128, 128, S], FP8, kind="ExternalInput")
    wq8_d = nc.dram_tensor("wq8", [HPC, 128, 8, 2, DK], FP8,
                           kind="ExternalInput")
    wk8_d = nc.dram_tensor("wk8", [HPC, 128, 8, 2, DK], FP8,
                           kind="ExternalInput")
    wvT_d = nc.dram_tensor("wvT", [128, D // 128, C], BF16,
                           kind="ExternalInput")
    woT_d = nc.dram_tensor("woT", [C, D], BF16, kind="ExternalInput")
    cs_d = nc.dram_tensor("cs", [128, 2, S], BF16, kind="ExternalInput")
    mask_d = nc.dram_tensor("maskbig", [128, 896], BF16, kind="ExternalInput")
    out_d = nc.dram_tensor("out", [S, D], BF16, kind="ExternalOutput")

    NT = D // 128         # 16 contraction tiles
    NQB = S // 512        # 4 q blocks

    with tile.TileContext(nc) as tc:
        with tc.tile_pool(name="persist", bufs=1) as pp:
            qts = [pp.tile([128, S], BF16, tag=f"qt{h}", name=f"qt{h}")
                   for h in range(HPC)]
            kts = [pp.tile([128, S], BF16, tag=f"kt{h}", name=f"kt{h}")
                   for h in range(HPC)]
            vt = pp.tile([128, NT, C], BF16, tag="vt")
            cxt = [pp.tile([128, S], BF16, tag=f"cx{h}", name=f"cx{h}")
                   for h in range(HPC)]
            cst = pp.tile([128, 2, S], BF16, tag="cst")
            msk = pp.tile([128, 896], BF16, tag="msk")
            zb = pp.tile([128, 1], F32, tag="zb")

            nc.vector.memset(zb[:], 0.0)

            def load_tables():
                nc.sync.dma_start(cst[:], cs_d.ap())
                nc.scalar.dma_start(msk[:], mask_d.ap())

            tbA = cst[:, 0, :]
            tbB = cst[:, 1, :]

            for _rep in range(repeat):
                with (
                    tc.tile_pool(name="rp", bufs=3) as rp,
                    tc.tile_pool(name="est", bufs=3) as estp,
                    tc.tile_pool(name="sm", bufs=4) as sm,
                ):
                    # phase A pools (projections) -> released before phase B
                    # (attention) opens its PSUM pools; LIFO stacks
                    es1 = ExitStack()
                    xw = es1.enter_context(tc.tile_pool(name="xw", bufs=3))
                    x8p = es1.enter_context(tc.tile_pool(name="x8p", bufs=8))
                    ws = es1.enter_context(tc.tile_pool(name="ws", bufs=2))
                    ps1 = es1.enter_context(
                        tc.tile_pool(name="ps1", bufs=1, space="PSUM"))
                    x8t = []     # per g: [half0 [128,2,1024], half1]
                    pstp = pavp = None   # phase B pools, assigned later

                    def rope_evict(ps, dest, js):
                        # dest = qsb*A + cross(qsb)*B with A=[cos;cos],
                        # B=[+sin;-sin]; cross-half reads pair same-base
                        # operands (walrus same-base rule for 2-SBUF ops).
                        # psum copy on ScalarE (idle in phase A) so it never
                        # queues behind the DVE rope muls
                        qsb = rp.tile([128, 512], BF16, tag="qsb", name="qsb")
                        nc.scalar.copy(qsb[:], ps[:])
                        nc.vector.tensor_mul(dest[:, js], qsb[:], tbA[:, js])
                        tb = rp.tile([128, 512], BF16, tag="tb", name="tb")
                        nc.vector.tensor_mul(tb[0:64, :], qsb[64:128, :],
                                             tbB[64:128, js])
                        nc.vector.tensor_mul(tb[64:128, :], qsb[0:64, :],
                                             tbB[0:64, js])
                        nc.vector.tensor_add(dest[:, js], dest[:, js], tb[:])

                    def load_x8_tile(g):
                        # one fp8 x tile [128, 2, 2048]; g pairs contraction
                        # tiles (2g, 2g+1); full seq width in one DMA
                        xt = x8p.tile([128, 2, S], FP8, tag="x8", name="x8")
                        xq = nc.scalar if g % 2 == 0 else nc.sync
                        xq.dma_start(
                            xt[:],
                            x8_d.ap()[2 * g:2 * g + 2, :, :]
                            .rearrange("j p n -> p j n"))
                        x8t.append(xt)

                    def qk_w_load(h):
                        # prefetch head h's Q/K fp8 weights (one DMA each)
                        wts = []
                        for which, wdram, xq in (("q", wq8_d, nc.sync),
                                                 ("k", wk8_d, nc.scalar)):
                            wt = ws.tile([128, 8, 2, DK], FP8,
                                         tag=f"w8{which}", name=f"w8{which}")
                            xq.dma_start(wt[:], wdram.ap()[h])
                            wts.append(wt)
                        return wts

                    def qk_dr(h, wts, inline_x=False):
                        # fp8 DoubleRow projection of head h: Q then K,
                        # J-blocks in pairs so evictions overlap matmuls;
                        # h=0 paces the x8 loads tile-by-tile with compute
                        for wi, (wt, dest) in enumerate(zip(wts, (qts, kts))):
                            for pair in range(2):
                                psums = [ps1.tile([128, 512], F32,
                                                  tag=f"pp{pair * 2 + i}",
                                                  name=f"pp{pair * 2 + i}")
                                         for i in range(2)]
                                for g in range(8):
                                    if inline_x and wi == 0 and pair == 0:
                                        load_x8_tile(g)
                                    for i in range(2):
                                        J = pair * 2 + i
                                        nc.tensor.matmul(
                                            psums[i][:],
                                            wt[:, g, :, :],
                                            x8t[g][:, :,
                                                   J * 512:(J + 1) * 512],
                                            start=(g == 0), stop=(g == 7),
                                            perf_mode=PM_DR,
                                        )
                                if inline_x and wi == 0 and pair == 0:
                                    load_tables()
                                for i in range(2):
                                    J = pair * 2 + i
                                    rope_evict(psums[i], dest[h],
                                               slice(J * 512, (J + 1) * 512))

                    def v_pass(lo):
                        # project V k-tiles lo..lo+3 (all heads); x streamed
                        # as [128, 512] column stripes, each used once
                        psums = [ps1.tile([128, 512], F32, tag=f"pv{i}",
                                          name=f"pv{i}") for i in range(4)]
                        for tg in range(NT // 4):
                            wt = ws.tile([128, 4, C], BF16, tag="wv",
                                         name="wv")
                            nc.sync.dma_start(
                                wt[:], wvT_d.ap()[:, tg * 4:(tg + 1) * 4, :])
                            # x stripe for 4 contraction tiles in one DMA
                            xv = xw.tile([128, 4, 512], BF16, tag="xv",
                                         name="xv")
                            xq = nc.scalar if tg % 2 == 0 else nc.sync
                            xq.dma_start(
                                xv[:],
                                xT_d.ap()[tg * 512:(tg + 1) * 512,
                                          lo * 128:(lo + 4) * 128]
                                .rearrange("(tt p) n -> p tt n", p=128))
                            for ti in range(4):
                                t = tg * 4 + ti
                                for i in range(4):
                                    nc.tensor.matmul(
                                        psums[i][:],
                                        xv[:, ti, i * 128:(i + 1) * 128],
                                        wt[:, ti, :],
                                        start=(t == 0),
                                        stop=(t == NT - 1),
                                    )
                        for i in range(4):
                            kt = lo + i
                            nc.scalar.copy(vt[:, kt, :], psums[i][:])

                    def attn_scores(h, J):
                        # scores + exp for block (h, J); exp runs on ScalarE
                        # while later-emitted PE work proceeds
                        nkt = 4 * J + 4
                        ests = []
                        for kt in range(nkt):
                            # diag tiles: only columns q >= s*128 are used
                            c0 = max(0, (kt - 4 * J)) * 128
                            pst = pstp.tile([128, 512], F32, tag="pst",
                                            name="pst")
                            nc.tensor.matmul(
                                pst[:, c0:512],
                                kts[h][:, kt * 128:(kt + 1) * 128],
                                qts[h][:, J * 512 + c0:(J + 1) * 512],
                                start=True, stop=True,
                            )
                            est = estp.tile([128, 512], BF16, tag=f"e{kt}",
                                            name=f"e{kt}")
                            if c0 > 0:
                                # below-diagonal region must be exact zeros:
                                # est tiles are consumed full-width now
                                nc.vector.memset(est[:, 0:c0], 0.0)
                            nc.scalar.activation(est[:, c0:512],
                                                 pst[:, c0:512], AF.Exp,
                                                 bias=zb[:])
                            if kt >= 4 * J:
                                # triangular mask on the 128-wide diag block
                                nc.vector.tensor_mul(
                                    est[:, c0:c0 + 128],
                                    est[:, c0:c0 + 128],
                                    msk[:, 384:512])
                            ests.append(est)
                        return ests

                    def attn_av(h, J, ests):
                        # ctxT accumulation with V stationary: cxt[h][:, Jq]
                        # = (sum_kt V_kt^T E_kt) / denom. Denominator =
                        # column sums of E: elementwise Esum on DVE, then
                        # partition-reduce on the otherwise-idle GpSimd.
                        nkt = 4 * J + 4
                        js = slice(J * 512, (J + 1) * 512)
                        pct = pavp.tile([128, 512], F32, tag="pct",
                                        name="pct")
                        for kt in range(nkt):
                            nc.tensor.matmul(
                                pct[:],
                                vt[:, kt, h * DK:(h + 1) * DK],
                                ests[kt][:],
                                start=(kt == 0),
                                stop=(kt == nkt - 1),
                            )
                        # Esum in bf16 on DVE: per-element rounding (~0.2%)
                        # averages out across the 128-partition reduce, so
                        # the denominator error lands at ~2e-4
                        esum = sm.tile([128, 512], BF16, tag="esum",
                                       name="esum", bufs=2)
                        nc.vector.tensor_add(esum[:], ests[0][:], ests[1][:])
                        for kt in range(2, nkt):
                            nc.vector.tensor_add(esum[:], esum[:],
                                                 ests[kt][:])
                        den = sm.tile([128, 512], F32, tag="den",
                                      name="den", bufs=2)
                        nc.gpsimd.partition_all_reduce(
                            den[:], esum[:], 128, bass_isa.ReduceOp.add)
                        rden = sm.tile([128, 512], F32, tag="rden",
                                       name="rden", bufs=2)
                        nc.vector.reciprocal(rden[:], den[:])
                        nc.vector.tensor_mul(cxt[h][:, js], pct[:], rden[:])

                    # ---- phase A: all projections (QK fp8-DR + V),
                    # interleaved so DMA paces under PE work ----
                    w_cur = qk_w_load(0)
                    qk_dr(0, w_cur, inline_x=True)
                    w_cur = qk_w_load(1)
                    v_pass(0)
                    qk_dr(1, w_cur)
                    w_cur = qk_w_load(2)
                    v_pass(4)
                    qk_dr(2, w_cur)
                    w_cur = qk_w_load(3)
                    v_pass(8)
                    qk_dr(3, w_cur)
                    v_pass(12)
                    es1.close()

                    # ---- phase B: attention rounds + output projection ----
                    with (
                        tc.tile_pool(name="pst", bufs=4,
                                     space="PSUM") as pstp_,
                        tc.tile_pool(name="pav", bufs=2,
                                     space="PSUM") as pavp_,
                        tc.tile_pool(name="pso", bufs=2, space="PSUM") as psop,
                        tc.tile_pool(name="lt", bufs=1) as ltp,
                        tc.tile_pool(name="ot", bufs=4) as otp,
                    ):
                        pstp, pavp = pstp_, pavp_
                        wot = ltp.tile([128, HPC, D], BF16, tag="wot")
                        wo_src = woT_d.ap().rearrange("(c p) e -> p c e",
                                                      p=128)
                        for ct in range(HPC):
                            oq = nc.sync if ct % 2 == 0 else nc.scalar
                            oq.dma_start(wot[:, ct, :], wo_src[:, ct, :])

                        def outproj_qt(qt):
                            # partial out rows qt*128.. for this core's
                            # heads; 4 psum evictions, one store DMA
                            ot = otp.tile([128, D], BF16, tag="ot",
                                          name="ot")
                            for eb in range(NQB):
                                pso = psop.tile([128, 512], F32,
                                                tag="pso", name="pso")
                                for ct in range(HPC):
                                    nc.tensor.matmul(
                                        pso[:],
                                        cxt[ct][:, qt * 128:(qt + 1) * 128],
                                        wot[:, ct, eb * 512:(eb + 1) * 512],
                                        start=(ct == 0),
                                        stop=(ct == HPC - 1),
                                    )
                                nc.vector.tensor_copy(
                                    ot[:, eb * 512:(eb + 1) * 512], pso[:])
                            oq = nc.sync if qt % 2 == 0 else nc.scalar
                            oq.dma_start(
                                out_d.ap()[qt * 128:(qt + 1) * 128, :],
                                ot[:])

                        # round 0: no outproj filler yet, pipeline depth 2
                        e0 = attn_scores(0, 0)
                        e1 = attn_scores(1, 0)
                        attn_av(0, 0, e0)
                        e2 = attn_scores(2, 0)
                        attn_av(1, 0, e1)
                        e3 = attn_scores(3, 0)
                        attn_av(2, 0, e2)
                        attn_av(3, 0, e3)
                        # rounds 1-3: depth 3 + outproj chunks as PE filler
                        for J in range(1, NQB):
                            qbase = (J - 1) * 4
                            pend_b = []
                            for h in range(HPC):
                                pend_b.append((h, attn_scores(h, J)))
                                outproj_qt(qbase + h)
                                if len(pend_b) > 2:
                                    hh, ee = pend_b.pop(0)
                                    attn_av(hh, J, ee)
                            for hh, ee in pend_b:
                                attn_av(hh, J, ee)
                        for qt in range(12, 16):
                            outproj_qt(qt)

    nc.compile()
    return nc


def get_nc():
    global _NC
    if _NC is None:
        _NC = _build_program()
    return _NC


def make_in_maps(x, wq, wk, wv, wo, token_positions):
    x = np.asarray(x, dtype=np.float32)
    wq = np.asarray(wq, dtype=np.float32)
    wk = np.asarray(wk, dtype=np.float32)
    wv = np.asarray(wv, dtype=np.float32)
    wo = np.asarray(wo, dtype=np.float32)
    pos = np.asarray(token_positions).astype(np.float64)

    bf = ml_dtypes.bfloat16
    f8 = ml_dtypes.float8_e4m3
    perm = np.concatenate([np.arange(0, DK, 2), np.arange(1, DK, 2)])
    WS = 2048.0              # fp8 weight scale (clears e4m3 denormal floor)
    f = DK ** -0.25 / WS     # undone at RoPE eviction via the cos/sin tables

    j = np.arange(DK // 2, dtype=np.float64)
    ang = pos[None, :] / (THETA ** (j[:, None] / (DK // 2)))
    cosv, sinv = np.cos(ang), np.sin(ang)
    A = np.concatenate([cosv, cosv], 0) * f        # [128, S]
    Bs = np.concatenate([sinv, -sinv], 0) * f      # [128, S]
    cs = np.ascontiguousarray(
        np.stack([A, Bs], 1)).astype(bf)           # [128, 2, S]

    kk = np.arange(128)[:, None]
    mm = np.arange(896)[None, :]
    maskbig = (mm >= kk + 384).astype(bf)
    xTb = [np.ascontiguousarray(x[b].T).astype(bf) for b in range(2)]
    x8b = [np.ascontiguousarray(x[b].T).reshape(16, 128, S).astype(f8)
           for b in range(2)]

    in_maps = []
    for core in range(NCORES):
        b, g = core // HPC, core % HPC
        rows = slice(g * C, (g + 1) * C)
        wq_s = wq[rows].reshape(HPC, DK, D)[:, perm].reshape(C, D) * WS
        wk_s = wk[rows].reshape(HPC, DK, D)[:, perm].reshape(C, D) * WS

        def tile_qk8(w_s):
            # [C, D] -> W.T [D, C] -> [h, p, g, j, c] for DoubleRow pairs
            wt = w_s.T.reshape(8, 2, 128, HPC, DK)
            return np.ascontiguousarray(wt.transpose(3, 2, 0, 1, 4)).astype(f8)

        wvt = wv[rows].T.reshape(16, 128, C)
        in_maps.append({
            "xT": xTb[b],
            "x8": x8b[b],
            "wq8": tile_qk8(wq_s),
            "wk8": tile_qk8(wk_s),
            "wvT": np.ascontiguousarray(wvt.transpose(1, 0, 2)).astype(bf),
            "woT": np.ascontiguousarray(wo[:, rows].T).astype(bf),
            "cs": cs,
            "maskbig": maskbig,
        })
    return in_maps


def kernel(x, wq, wk, wv, wo, token_positions):
    nc = get_nc()
    in_maps = make_in_maps(x, wq, wk, wv, wo, token_positions)
    res = bass_utils.run_bass_kernel_spmd(
        nc, in_maps, core_ids=list(range(NCORES)))
    out = np.zeros((2, S, D), dtype=np.float32)
    for core in range(NCORES):
        out[core // HPC] += res.results[core]["out"]
    return out



# revision 60
# speedup vs baseline: 1.0593x; 1.0593x over previous
"""Trainium2 Bass kernel for multi-head self-attention with RoPE.

Problem: x[2,2048,2048] f32, Wq/Wk/Wv/Wo [2048,2048], causal MHA, 16 heads,
dk=128, RoPE on Q/K. Sharding: 8 cores = 2 batches x 4 head-groups
(4 heads/core). Each core computes its batch's partial output projection for
its 4 heads; host sums the 4 partials per batch.

Device-side scheme (per core, all matmuls bf16 with f32 PSUM accumulation):
  - host pre-transposes x -> xT [D,S] and weight slices; RoPE pair
    de-interleave is folded into a row permutation of Wq/Wk so the rotation
    becomes partition-block ops; 1/sqrt(dk) folded into Wq/Wk.
  - QT/KT [dk,S] = W-slice^T-tiles @ xT-tiles (+RoPE, bf16 vector ops)
  - V [S,dk] with a ones column appended (interleaved [.,516] layout)
  - ST tiles [k,q] = KT-slice^T @ QT; exp on ScalarE; causal mask only on
    diagonal tiles (multiply by precomputed triangular mask)
  - ctx [q, dk+1] = expST^T @ V_aug; column dk holds the softmax denominator
  - ctx scaled by 1/denom during PSUM eviction, transposed via PE to ctxT
  - out [S, E] = ctxT^T @ WoT-slice, accumulated over the 4 head-chunks
Phases are emitted interleaved (projection passes between attention blocks)
so projection matmuls fill PE gaps while ScalarE computes exp.
"""
from contextlib import ExitStack

import numpy as np
import ml_dtypes

try:
    import concourse.bass as bass  # noqa: F401
except ImportError:  # fresh grading dir: repo lives at /opt/trn_rl_repo
    import sys
    sys.path.insert(0, "/opt/trn_rl_repo")

import concourse.bass as bass
import concourse.bass_isa as bass_isa
import concourse.mybir as mybir
import concourse.tile as tile
from concourse import bacc, bass_utils

BF16 = mybir.dt.bfloat16
F32 = mybir.dt.float32
FP8 = mybir.dt.float8e4
PM_DR = mybir.MatmulPerfMode.DoubleRow
AF = mybir.ActivationFunctionType

D = 2048          # model dim
S = 2048          # sequence length
DK = 128          # head dim
HPC = 4           # heads per core
C = HPC * DK      # per-core feature slice = 512
THETA = 10000.0
NCORES = 8

_NC = None  # cached compiled Bass module


def _build_program(repeat=1):
    nc = bacc.Bacc("TRN2", debug=False, num_devices=NCORES)

    xT_d = nc.dram_tensor("xT", [D, S], BF16, kind="ExternalInput")
    # fp8 copies for the Q/K projections (DoubleRow): x8[t, p, s] = xT[t*128+p, s];
    # wq8/wk8[h, p, g, j, c] = (WqT_perm * 2048)[(2g+j)*128+p, h*DK+c]
    x8_d = nc.dram_tensor("x8", [D // 128, 128, S], FP8, kind="ExternalInput")
    wq8_d = nc.dram_tensor("wq8", [HPC, 128, 8, 2, DK], FP8,
                           kind="ExternalInput")
    wk8_d = nc.dram_tensor("wk8", [HPC, 128, 8, 2, DK], FP8,
                           kind="ExternalInput")
    wvT_d = nc.dram_tensor("wvT", [128, D // 128, C], BF16,
                           kind="ExternalInput")
    woT_d = nc.dram_tensor("woT", [C, D], BF16, kind="ExternalInput")
    cs_d = nc.dram_tensor("cs", [128, 2, S], BF16, kind="ExternalInput")
    mask_d = nc.dram_tensor("maskbig", [128, 896], BF16, kind="ExternalInput")
    out_d = nc.dram_tensor("out", [S, D], BF16, kind="ExternalOutput")

    NT = D // 128         # 16 contraction tiles
    NQB = S // 512        # 4 q blocks

    with tile.TileContext(nc) as tc:
        with tc.tile_pool(name="persist", bufs=1) as pp:
            qts = [pp.tile([128, S], BF16, tag=f"qt{h}", name=f"qt{h}")
                   for h in range(HPC)]
            kts = [pp.tile([128, S], BF16, tag=f"kt{h}", name=f"kt{h}")
                   for h in range(HPC)]
            vt = pp.tile([128, NT, C], BF16, tag="vt")
            cxt = [pp.tile([128, S], BF16, tag=f"cx{h}", name=f"cx{h}")
                   for h in range(HPC)]
            cst = pp.tile([128, 2, S], BF16, tag="cst")
            msk = pp.tile([128, 896], BF16, tag="msk")
            zb = pp.tile([128, 1], F32, tag="zb")

            nc.vector.memset(zb[:], 0.0)

            def load_tables():
                nc.sync.dma_start(cst[:], cs_d.ap())
                nc.scalar.dma_start(msk[:], mask_d.ap())

            tbA = cst[:, 0, :]
            tbB = cst[:, 1, :]

            for _rep in range(repeat):
                with ExitStack() as es0:
                    # phase A pools (projections) -> released before phase B
                    # (attention) opens its pools; LIFO stacks
                    es1 = ExitStack()
                    rp = es1.enter_context(tc.tile_pool(name="rp", bufs=3))
                    xw = es1.enter_context(tc.tile_pool(name="xw", bufs=3))
                    x8p = es1.enter_context(tc.tile_pool(name="x8p", bufs=8))
                    ws = es1.enter_context(tc.tile_pool(name="ws", bufs=2))
                    ps1 = es1.enter_context(
                        tc.tile_pool(name="ps1", bufs=1, space="PSUM"))
                    x8t = []     # per g: fp8 x tile [128, 2, 2048]
                    estp = sm = pstp = pavp = None   # phase B, see below

                    def rope_evict(ps, dest, js):
                        # dest = qsb*A + cross(qsb)*B with A=[cos;cos],
                        # B=[+sin;-sin]; cross-half reads pair same-base
                        # operands (walrus same-base rule for 2-SBUF ops).
                        # psum copy on ScalarE (idle in phase A) so it never
                        # queues behind the DVE rope muls
                        qsb = rp.tile([128, 512], BF16, tag="qsb", name="qsb")
                        nc.scalar.copy(qsb[:], ps[:])
                        nc.vector.tensor_mul(dest[:, js], qsb[:], tbA[:, js])
                        tb = rp.tile([128, 512], BF16, tag="tb", name="tb")
                        nc.vector.tensor_mul(tb[0:64, :], qsb[64:128, :],
                                             tbB[64:128, js])
                        nc.vector.tensor_mul(tb[64:128, :], qsb[0:64, :],
                                             tbB[0:64, js])
                        nc.vector.tensor_add(dest[:, js], dest[:, js], tb[:])

                    def load_x8_tile(g):
                        # one fp8 x tile [128, 2, 2048]; g pairs contraction
                        # tiles (2g, 2g+1); full seq width in one DMA
                        xt = x8p.tile([128, 2, S], FP8, tag="x8", name="x8")
                        xq = nc.scalar if g % 2 == 0 else nc.sync
                        xq.dma_start(
                            xt[:],
                            x8_d.ap()[2 * g:2 * g + 2, :, :]
                            .rearrange("j p n -> p j n"))
                        x8t.append(xt)

                    def qk_w_load(h):
                        # prefetch head h's Q/K fp8 weights (one DMA each)
                        wts = []
                        for which, wdram, xq in (("q", wq8_d, nc.sync),
                                                 ("k", wk8_d, nc.scalar)):
                            wt = ws.tile([128, 8, 2, DK], FP8,
                                         tag=f"w8{which}", name=f"w8{which}")
                            xq.dma_start(wt[:], wdram.ap()[h])
                            wts.append(wt)
                        return wts

                    def qk_dr(h, wts, inline_x=False):
                        # fp8 DoubleRow projection of head h: Q then K,
                        # J-blocks in pairs so evictions overlap matmuls;
                        # h=0 paces the x8 loads tile-by-tile with compute
                        for wi, (wt, dest) in enumerate(zip(wts, (qts, kts))):
                            for pair in range(2):
                                psums = [ps1.tile([128, 512], F32,
                                                  tag=f"pp{pair * 2 + i}",
                                                  name=f"pp{pair * 2 + i}")
                                         for i in range(2)]
                                for g in range(8):
                                    if inline_x and wi == 0 and pair == 0:
                                        load_x8_tile(g)
                                    for i in range(2):
                                        J = pair * 2 + i
                                        nc.tensor.matmul(
                                            psums[i][:],
                                            wt[:, g, :, :],
                                            x8t[g][:, :,
                                                   J * 512:(J + 1) * 512],
                                            start=(g == 0), stop=(g == 7),
                                            perf_mode=PM_DR,
                                        )
                                if inline_x and wi == 0 and pair == 0:
                                    load_tables()
                                for i in range(2):
                                    J = pair * 2 + i
                                    rope_evict(psums[i], dest[h],
                                               slice(J * 512, (J + 1) * 512))

                    def v_pass(lo):
                        # project V k-tiles lo..lo+3 (all heads); x streamed
                        # as [128, 512] column stripes, each used once
                        psums = [ps1.tile([128, 512], F32, tag=f"pv{i}",
                                          name=f"pv{i}") for i in range(4)]
                        for tg in range(NT // 4):
                            wt = ws.tile([128, 4, C], BF16, tag="wv",
                                         name="wv")
                            nc.sync.dma_start(
                                wt[:], wvT_d.ap()[:, tg * 4:(tg + 1) * 4, :])
                            # x stripe for 4 contraction tiles in one DMA
                            xv = xw.tile([128, 4, 512], BF16, tag="xv",
                                         name="xv")
                            xq = nc.scalar if tg % 2 == 0 else nc.sync
                            xq.dma_start(
                                xv[:],
                                xT_d.ap()[tg * 512:(tg + 1) * 512,
                                          lo * 128:(lo + 4) * 128]
                                .rearrange("(tt p) n -> p tt n", p=128))
                            for ti in range(4):
                                t = tg * 4 + ti
                                for i in range(4):
                                    nc.tensor.matmul(
                                        psums[i][:],
                                        xv[:, ti, i * 128:(i + 1) * 128],
                                        wt[:, ti, :],
                                        start=(t == 0),
                                        stop=(t == NT - 1),
                                    )
                        for i in range(4):
                            kt = lo + i
                            nc.scalar.copy(vt[:, kt, :], psums[i][:])

                    def attn_scores(h, J):
                        # scores + exp for block (h, J); exp runs on ScalarE
                        # while later-emitted PE work proceeds
                        nkt = 4 * J + 4
                        ests = []
                        for kt in range(nkt):
                            # diag tiles: only columns q >= s*128 are used
                            c0 = max(0, (kt - 4 * J)) * 128
                            pst = pstp.tile([128, 512], F32, tag="pst",
                                            name="pst")
                            nc.tensor.matmul(
                                pst[:, c0:512],
                                kts[h][:, kt * 128:(kt + 1) * 128],
                                qts[h][:, J * 512 + c0:(J + 1) * 512],
                                start=True, stop=True,
                            )
                            est = estp.tile([128, 512], BF16, tag=f"e{kt}",
                                            name=f"e{kt}")
                            if c0 > 0:
                                # below-diagonal region must be exact zeros:
                                # est tiles are consumed full-width now
                                nc.any.memset(est[:, 0:c0], 0.0)
                            nc.scalar.activation(est[:, c0:512],
                                                 pst[:, c0:512], AF.Exp,
                                                 bias=zb[:])
                            if kt >= 4 * J:
                                # triangular mask on the 128-wide diag block
                                nc.vector.tensor_mul(
                                    est[:, c0:c0 + 128],
                                    est[:, c0:c0 + 128],
                                    msk[:, 384:512])
                            ests.append(est)
                        return ests

                    def attn_av(h, J, ests):
                        # ctxT accumulation with V stationary: cxt[h][:, Jq]
                        # = (sum_kt V_kt^T E_kt) / denom. Denominator =
                        # column sums of E: elementwise Esum on DVE, then
                        # partition-reduce on the otherwise-idle GpSimd.
                        nkt = 4 * J + 4
                        js = slice(J * 512, (J + 1) * 512)
                        # Esum in bf16 on DVE as 4 parallel chains + merge:
                        # keeps the dependent-op depth ~nkt/4+2 so DVE
                        # latency stays off the critical path. bf16 rounding
                        # (~0.2%/elem) averages out across the 128-partition
                        # reduce (~2e-4 on the denominator).
                        esum = sm.tile([128, 512], BF16, tag="esum",
                                       name="esum", bufs=2)
                        if nkt == 4:
                            m0 = sm.tile([128, 512], BF16, tag="m0",
                                         name="m0", bufs=2)
                            m1 = sm.tile([128, 512], BF16, tag="m1",
                                         name="m1", bufs=2)
                            nc.vector.tensor_add(m0[:], ests[0][:],
                                                 ests[1][:])
                            nc.vector.tensor_add(m1[:], ests[2][:],
                                                 ests[3][:])
                            nc.vector.tensor_add(esum[:], m0[:], m1[:])
                        else:
                            chains = []
                            for cc in range(4):
                                idxs = list(range(cc, nkt, 4))
                                t = sm.tile([128, 512], BF16, tag=f"ch{cc}",
                                            name=f"ch{cc}", bufs=2)
                                nc.vector.tensor_add(t[:], ests[idxs[0]][:],
                                                     ests[idxs[1]][:])
                                for kt in idxs[2:]:
                                    nc.vector.tensor_add(t[:], t[:],
                                                         ests[kt][:])
                                chains.append(t)
                            m0 = sm.tile([128, 512], BF16, tag="m0",
                                         name="m0", bufs=2)
                            m1 = sm.tile([128, 512], BF16, tag="m1",
                                         name="m1", bufs=2)
                            nc.vector.tensor_add(m0[:], chains[0][:],
                                                 chains[1][:])
                            nc.vector.tensor_add(m1[:], chains[2][:],
                                                 chains[3][:])
                            nc.vector.tensor_add(esum[:], m0[:], m1[:])
                        den = sm.tile([128, 512], F32, tag="den",
                                      name="den", bufs=2)
                        nc.gpsimd.partition_all_reduce(
                            den[:], esum[:], 128, bass_isa.ReduceOp.add)
                        rden = sm.tile([128, 512], F32, tag="rden",
                                       name="rden", bufs=2)
                        nc.vector.reciprocal(rden[:], den[:])
                        # pct matmuls emitted after the denominator chain so
                        # DVE/GpSimd work overlaps the PE accumulation
                        pct = pavp.tile([128, 512], F32, tag="pct",
                                        name="pct")
                        for kt in range(nkt):
                            nc.tensor.matmul(
                                pct[:],
                                vt[:, kt, h * DK:(h + 1) * DK],
                                ests[kt][:],
                                start=(kt == 0),
                                stop=(kt == nkt - 1),
                            )
                        nc.vector.tensor_mul(cxt[h][:, js], pct[:], rden[:])

                    # ---- phase A: all projections (QK fp8-DR + V),
                    # interleaved so DMA paces under PE work ----
                    w_cur = qk_w_load(0)
                    qk_dr(0, w_cur, inline_x=True)
                    w_cur = qk_w_load(1)
                    qk_dr(1, w_cur)
                    v_pass(0)
                    w_cur = qk_w_load(2)
                    qk_dr(2, w_cur)
                    v_pass(4)
                    w_cur = qk_w_load(3)
                    qk_dr(3, w_cur)
                    v_pass(8)
                    v_pass(12)
                    es1.close()

                    # ---- phase B: attention rounds + output projection ----
                    with (
                        tc.tile_pool(name="est", bufs=4) as estp_,
                        tc.tile_pool(name="sm", bufs=4) as sm_,
                        tc.tile_pool(name="pst", bufs=4,
                                     space="PSUM") as pstp_,
                        tc.tile_pool(name="pav", bufs=2,
                                     space="PSUM") as pavp_,
                        tc.tile_pool(name="pso", bufs=2, space="PSUM") as psop,
                        tc.tile_pool(name="lt", bufs=1) as ltp,
                        tc.tile_pool(name="ot", bufs=4) as otp,
                    ):
                        estp, sm, pstp, pavp = estp_, sm_, pstp_, pavp_
                        wot = ltp.tile([128, HPC, D], BF16, tag="wot")
                        wo_src = woT_d.ap().rearrange("(c p) e -> p c e",
                                                      p=128)
                        for ct in range(HPC):
                            oq = nc.sync if ct % 2 == 0 else nc.scalar
                            oq.dma_start(wot[:, ct, :], wo_src[:, ct, :])

                        def outproj_qt(qt):
                            # partial out rows qt*128.. for this core's
                            # heads; 4 psum evictions, one store DMA
                            ot = otp.tile([128, D], BF16, tag="ot",
                                          name="ot")
                            for eb in range(NQB):
                                pso = psop.tile([128, 512], F32,
                                                tag="pso", name="pso")
                                for ct in range(HPC):
                                    nc.tensor.matmul(
                                        pso[:],
                                        cxt[ct][:, qt * 128:(qt + 1) * 128],
                                        wot[:, ct, eb * 512:(eb + 1) * 512],
                                        start=(ct == 0),
                                        stop=(ct == HPC - 1),
                                    )
                                nc.vector.tensor_copy(
                                    ot[:, eb * 512:(eb + 1) * 512], pso[:])
                            oq = nc.sync if qt % 2 == 0 else nc.scalar
                            oq.dma_start(
                                out_d.ap()[qt * 128:(qt + 1) * 128, :],
                                ot[:])

                        # round 0: no outproj filler yet, pipeline depth 2
                        e0 = attn_scores(0, 0)
                        e1 = attn_scores(1, 0)
                        attn_av(0, 0, e0)
                        e2 = attn_scores(2, 0)
                        attn_av(1, 0, e1)
                        e3 = attn_scores(3, 0)
                        attn_av(2, 0, e2)
                        attn_av(3, 0, e3)
                        # rounds 1-3: depth 3 + outproj chunks as PE filler
                        for J in range(1, NQB):
                            qbase = (J - 1) * 4
                            pend_b = []
                            for h in range(HPC):
                                pend_b.append((h, attn_scores(h, J)))
                                outproj_qt(qbase + h)
                                if len(pend_b) > 2:
                                    hh, ee = pend_b.pop(0)
                                    attn_av(hh, J, ee)
                            for hh, ee in pend_b:
                                attn_av(hh, J, ee)
                        for qt in range(12, 16):
                            outproj_qt(qt)

    nc.compile()
    return nc


def get_nc():
    global _NC
    if _NC is None:
        _NC = _build_program()
    return _NC


def make_in_maps(x, wq, wk, wv, wo, token_positions):
    x = np.asarray(x, dtype=np.float32)
    wq = np.asarray(wq, dtype=np.float32)
    wk = np.asarray(wk, dtype=np.float32)
    wv = np.asarray(wv, dtype=np.float32)
    wo = np.asarray(wo, dtype=np.float32)
    pos = np.asarray(token_positions).astype(np.float64)

    bf = ml_dtypes.bfloat16
    f8 = ml_dtypes.float8_e4m3
    perm = np.concatenate([np.arange(0, DK, 2), np.arange(1, DK, 2)])
    WS = 2048.0              # fp8 weight scale (clears e4m3 denormal floor)
    f = DK ** -0.25 / WS     # undone at RoPE eviction via the cos/sin tables

    j = np.arange(DK // 2, dtype=np.float64)
    ang = pos[None, :] / (THETA ** (j[:, None] / (DK // 2)))
    cosv, sinv = np.cos(ang), np.sin(ang)
    A = np.concatenate([cosv, cosv], 0) * f        # [128, S]
    Bs = np.concatenate([sinv, -sinv], 0) * f      # [128, S]
    cs = np.ascontiguousarray(
        np.stack([A, Bs], 1)).astype(bf)           # [128, 2, S]

    kk = np.arange(128)[:, None]
    mm = np.arange(896)[None, :]
    maskbig = (mm >= kk + 384).astype(bf)
    xTb = [np.ascontiguousarray(x[b].T).astype(bf) for b in range(2)]
    x8b = [np.ascontiguousarray(x[b].T).reshape(16, 128, S).astype(f8)
           for b in range(2)]

    in_maps = []
    for core in range(NCORES):
        b, g = core // HPC, core % HPC
        rows = slice(g * C, (g + 1) * C)
        wq_s = wq[rows].reshape(HPC, DK, D)[:, perm].reshape(C, D) * WS
        wk_s = wk[rows].reshape(HPC, DK, D)[:, perm].reshape(C, D) * WS

        def tile_qk8(w_s):
            # [C, D] -> W.T [D, C] -> [h, p, g, j, c] for DoubleRow pairs
            wt = w_s.T.reshape(8, 2, 128, HPC, DK)
            return np.ascontiguousarray(wt.transpose(3, 2, 0, 1, 4)).astype(f8)

        wvt = wv[rows].T.reshape(16, 128, C)
        in_maps.append({
            "xT": xTb[b],
            "x8": x8b[b],
            "wq8": tile_qk8(wq_s),
            "wk8": tile_qk8(wk_s),
            "wvT": np.ascontiguousarray(wvt.transpose(1, 0, 2)).astype(bf),
            "woT": np.ascontiguousarray(wo[:, rows].T).astype(bf),
            "cs": cs,
            "maskbig": maskbig,
        })
    return in_maps


def kernel(x, wq, wk, wv, wo, token_positions):
    nc = get_nc()
    in_maps = make_in_maps(x, wq, wk, wv, wo, token_positions)
    res = bass_utils.run_bass_kernel_spmd(
        nc, in_maps, core_ids=list(range(NCORES)))
    out = np.zeros((2, S, D), dtype=np.float32)
    for core in range(NCORES):
        out[core // HPC] += res.results[core]["out"]
    return out



# revision 70
# speedup vs baseline: 1.1138x; 1.0514x over previous
"""Trainium2 Bass kernel for multi-head self-attention with RoPE.

Problem: x[2,2048,2048] f32, Wq/Wk/Wv/Wo [2048,2048], causal MHA, 16 heads,
dk=128, RoPE on Q/K. Sharding: 8 cores = 2 batches x 4 head-groups
(4 heads/core). Each core computes its batch's partial output projection for
its 4 heads; host sums the 4 partials per batch.

Device-side scheme (per core, all matmuls bf16 with f32 PSUM accumulation):
  - host pre-transposes x -> xT [D,S] and weight slices; RoPE pair
    de-interleave is folded into a row permutation of Wq/Wk so the rotation
    becomes partition-block ops; 1/sqrt(dk) folded into Wq/Wk.
  - QT/KT [dk,S] = W-slice^T-tiles @ xT-tiles (+RoPE, bf16 vector ops)
  - V [S,dk] with a ones column appended (interleaved [.,516] layout)
  - ST tiles [k,q] = KT-slice^T @ QT; exp on ScalarE; causal mask only on
    diagonal tiles (multiply by precomputed triangular mask)
  - ctx [q, dk+1] = expST^T @ V_aug; column dk holds the softmax denominator
  - ctx scaled by 1/denom during PSUM eviction, transposed via PE to ctxT
  - out [S, E] = ctxT^T @ WoT-slice, accumulated over the 4 head-chunks
Phases are emitted interleaved (projection passes between attention blocks)
so projection matmuls fill PE gaps while ScalarE computes exp.
"""
from contextlib import ExitStack

import numpy as np
import ml_dtypes

try:
    import concourse.bass as bass  # noqa: F401
except ImportError:  # fresh grading dir: repo lives at /opt/trn_rl_repo
    import sys
    sys.path.insert(0, "/opt/trn_rl_repo")

import concourse.bass as bass
import concourse.bass_isa as bass_isa
import concourse.mybir as mybir
import concourse.tile as tile
from concourse import bacc, bass_utils

BF16 = mybir.dt.bfloat16
F32 = mybir.dt.float32
FP8 = mybir.dt.float8e4
PM_DR = mybir.MatmulPerfMode.DoubleRow
AF = mybir.ActivationFunctionType

D = 2048          # model dim
S = 2048          # sequence length
DK = 128          # head dim
HPC = 4           # heads per core
C = HPC * DK      # per-core feature slice = 512
THETA = 10000.0
NCORES = 8

_NC = None  # cached compiled Bass module


def _build_program(repeat=1):
    nc = bacc.Bacc("TRN2", debug=False, num_devices=NCORES)

    xT_d = nc.dram_tensor("xT", [D, S], BF16, kind="ExternalInput")
    # fp8 copies for the Q/K projections (DoubleRow): x8[t, p, s] = xT[t*128+p, s];
    # wq8/wk8[h, p, g, j, c] = (WqT_perm * 2048)[(2g+j)*128+p, h*DK+c]
    x8_d = nc.dram_tensor("x8", [D // 128, 128, S], FP8, kind="ExternalInput")
    wq8_d = nc.dram_tensor("wq8", [HPC, 128, 8, 2, DK], FP8,
                           kind="ExternalInput")
    wk8_d = nc.dram_tensor("wk8", [HPC, 128, 8, 2, DK], FP8,
                           kind="ExternalInput")
    wvT_d = nc.dram_tensor("wvT", [128, D // 128, C], BF16,
                           kind="ExternalInput")
    woT_d = nc.dram_tensor("woT", [C, D], BF16, kind="ExternalInput")
    cs_d = nc.dram_tensor("cs", [128, 2, S], BF16, kind="ExternalInput")
    mask_d = nc.dram_tensor("maskbig", [128, 896], BF16, kind="ExternalInput")
    out_d = nc.dram_tensor("out", [S, D], BF16, kind="ExternalOutput")

    NT = D // 128         # 16 contraction tiles
    NQB = S // 512        # 4 q blocks

    with tile.TileContext(nc) as tc:
        with tc.tile_pool(name="persist", bufs=1) as pp:
            qts = [pp.tile([128, S], BF16, tag=f"qt{h}", name=f"qt{h}")
                   for h in range(HPC)]
            kts = [pp.tile([128, S], BF16, tag=f"kt{h}", name=f"kt{h}")
                   for h in range(HPC)]
            vt = pp.tile([128, NT, C], BF16, tag="vt")
            cxt = [pp.tile([128, S], BF16, tag=f"cx{h}", name=f"cx{h}")
                   for h in range(HPC)]
            cst = pp.tile([128, 2, S], BF16, tag="cst")
            msk = pp.tile([128, 896], BF16, tag="msk")
            zb = pp.tile([128, 1], F32, tag="zb")
            onec = pp.tile([128, 1], BF16, tag="onec")
            oner = pp.tile([1, 128], BF16, tag="oner")

            nc.vector.memset(zb[:], 0.0)
            nc.vector.memset(onec[:], 1.0)
            nc.vector.memset(oner[:], 1.0)

            def load_tables():
                nc.sync.dma_start(cst[:], cs_d.ap())
                nc.scalar.dma_start(msk[:], mask_d.ap())

            tbA = cst[:, 0, :]
            tbB = cst[:, 1, :]

            for _rep in range(repeat):
                with ExitStack() as es0:
                    # phase A pools (projections) -> released before phase B
                    # (attention) opens its pools; LIFO stacks
                    es1 = ExitStack()
                    rp = es1.enter_context(tc.tile_pool(name="rp", bufs=3))
                    xw = es1.enter_context(tc.tile_pool(name="xw", bufs=3))
                    x8p = es1.enter_context(tc.tile_pool(name="x8p", bufs=8))
                    ws = es1.enter_context(tc.tile_pool(name="ws", bufs=2))
                    ps1 = es1.enter_context(
                        tc.tile_pool(name="ps1", bufs=1, space="PSUM"))
                    x8t = []     # per g: fp8 x tile [128, 2, 2048]
                    estp = sm = pstp = pavp = None   # phase B, see below

                    def rope_evict(ps, dest, js):
                        # dest = qsb*A + cross(qsb)*B with A=[cos;cos],
                        # B=[+sin;-sin]; cross-half reads pair same-base
                        # operands (walrus same-base rule for 2-SBUF ops).
                        # psum copy on ScalarE (idle in phase A) so it never
                        # queues behind the DVE rope muls
                        qsb = rp.tile([128, 512], BF16, tag="qsb", name="qsb")
                        nc.scalar.copy(qsb[:], ps[:])
                        nc.vector.tensor_mul(dest[:, js], qsb[:], tbA[:, js])
                        tb = rp.tile([128, 512], BF16, tag="tb", name="tb")
                        nc.vector.tensor_mul(tb[0:64, :], qsb[64:128, :],
                                             tbB[64:128, js])
                        nc.vector.tensor_mul(tb[64:128, :], qsb[0:64, :],
                                             tbB[0:64, js])
                        nc.vector.tensor_add(dest[:, js], dest[:, js], tb[:])

                    def load_x8_tile(g):
                        # one fp8 x tile [128, 2, 2048]; g pairs contraction
                        # tiles (2g, 2g+1); full seq width in one DMA
                        xt = x8p.tile([128, 2, S], FP8, tag="x8", name="x8")
                        xq = nc.scalar if g % 2 == 0 else nc.sync
                        xq.dma_start(
                            xt[:],
                            x8_d.ap()[2 * g:2 * g + 2, :, :]
                            .rearrange("j p n -> p j n"))
                        x8t.append(xt)

                    def qk_w_load(h):
                        # prefetch head h's Q/K fp8 weights (one DMA each)
                        wts = []
                        for which, wdram, xq in (("q", wq8_d, nc.sync),
                                                 ("k", wk8_d, nc.scalar)):
                            wt = ws.tile([128, 8, 2, DK], FP8,
                                         tag=f"w8{which}", name=f"w8{which}")
                            xq.dma_start(wt[:], wdram.ap()[h])
                            wts.append(wt)
                        return wts

                    def qk_dr(h, wts, inline_x=False):
                        # fp8 DoubleRow projection of head h: Q then K,
                        # J-blocks in pairs so evictions overlap matmuls;
                        # h=0 paces the x8 loads tile-by-tile with compute
                        for wi, (wt, dest) in enumerate(zip(wts, (qts, kts))):
                            for pair in range(2):
                                psums = [ps1.tile([128, 512], F32,
                                                  tag=f"pp{pair * 2 + i}",
                                                  name=f"pp{pair * 2 + i}")
                                         for i in range(2)]
                                for g in range(8):
                                    if inline_x and wi == 0 and pair == 0:
                                        load_x8_tile(g)
                                    for i in range(2):
                                        J = pair * 2 + i
                                        nc.tensor.matmul(
                                            psums[i][:],
                                            wt[:, g, :, :],
                                            x8t[g][:, :,
                                                   J * 512:(J + 1) * 512],
                                            start=(g == 0), stop=(g == 7),
                                            perf_mode=PM_DR,
                                        )
                                if inline_x and wi == 0 and pair == 0:
                                    load_tables()
                                for i in range(2):
                                    J = pair * 2 + i
                                    rope_evict(psums[i], dest[h],
                                               slice(J * 512, (J + 1) * 512))

                    def v_pass(lo):
                        # project V k-tiles lo..lo+3 (all heads); x streamed
                        # as [128, 512] column stripes, each used once
                        psums = [ps1.tile([128, 512], F32, tag=f"pv{i}",
                                          name=f"pv{i}") for i in range(4)]
                        for tg in range(NT // 4):
                            wt = ws.tile([128, 4, C], BF16, tag="wv",
                                         name="wv")
                            nc.sync.dma_start(
                                wt[:], wvT_d.ap()[:, tg * 4:(tg + 1) * 4, :])
                            # x stripe for 4 contraction tiles in one DMA
                            xv = xw.tile([128, 4, 512], BF16, tag="xv",
                                         name="xv")
                            xq = nc.scalar if tg % 2 == 0 else nc.sync
                            xq.dma_start(
                                xv[:],
                                xT_d.ap()[tg * 512:(tg + 1) * 512,
                                          lo * 128:(lo + 4) * 128]
                                .rearrange("(tt p) n -> p tt n", p=128))
                            for ti in range(4):
                                t = tg * 4 + ti
                                for i in range(4):
                                    nc.tensor.matmul(
                                        psums[i][:],
                                        xv[:, ti, i * 128:(i + 1) * 128],
                                        wt[:, ti, :],
                                        start=(t == 0),
                                        stop=(t == NT - 1),
                                    )
                        for i in range(4):
                            kt = lo + i
                            nc.scalar.copy(vt[:, kt, :], psums[i][:])

                    def attn_scores(h, J):
                        # scores + exp for block (h, J); exp runs on ScalarE
                        # while later-emitted PE work proceeds
                        nkt = 4 * J + 4
                        ests = []
                        for kt in range(nkt):
                            # diag tiles: only columns q >= s*128 are used
                            c0 = max(0, (kt - 4 * J)) * 128
                            pst = pstp.tile([128, 512], F32, tag="pst",
                                            name="pst")
                            nc.tensor.matmul(
                                pst[:, c0:512],
                                kts[h][:, kt * 128:(kt + 1) * 128],
                                qts[h][:, J * 512 + c0:(J + 1) * 512],
                                start=True, stop=True,
                            )
                            est = estp.tile([128, 512], BF16, tag=f"e{kt}",
                                            name=f"e{kt}")
                            if c0 > 0:
                                # below-diagonal region must be exact zeros:
                                # est tiles are consumed full-width now
                                nc.any.memset(est[:, 0:c0], 0.0)
                            nc.scalar.activation(est[:, c0:512],
                                                 pst[:, c0:512], AF.Exp,
                                                 bias=zb[:])
                            if kt >= 4 * J:
                                # triangular mask on the 128-wide diag block
                                nc.vector.tensor_mul(
                                    est[:, c0:c0 + 128],
                                    est[:, c0:c0 + 128],
                                    msk[:, 384:512])
                            ests.append(est)
                        return ests

                    pend_av = []

                    def flush_av():
                        # finish a previous block: broadcast 1/den via a K=1
                        # matmul and scale the ctxT accumulator into cxt
                        if pend_av:
                            hh, jjs, pct0, rd0 = pend_av.pop()
                            rdb = dnp.tile([128, 512], F32, tag="dn",
                                           name="rdb")
                            nc.tensor.matmul(rdb[:], oner[:], rd0[:],
                                             start=True, stop=True)
                            # DVE can't read two PSUM operands: stage the
                            # broadcast through SBUF on ScalarE
                            rdbs = sm.tile([128, 512], F32, tag="rdbs",
                                           name="rdbs", bufs=2)
                            nc.scalar.copy(rdbs[:], rdb[:])
                            nc.vector.tensor_mul(cxt[hh][:, jjs], pct0[:],
                                                 rdbs[:])

                    def attn_av(h, J, ests):
                        # ctxT accumulation with V stationary: cxt[h][:, Jq]
                        # = (sum_kt V_kt^T E_kt) / denom. Denominator =
                        # column sums of E: elementwise Esum on DVE, then
                        # partition-reduce on the otherwise-idle GpSimd.
                        nkt = 4 * J + 4
                        js = slice(J * 512, (J + 1) * 512)
                        # Esum in bf16 on DVE as 4 parallel chains + merge:
                        # keeps the dependent-op depth ~nkt/4+2 so DVE
                        # latency stays off the critical path. bf16 rounding
                        # (~0.2%/elem) averages out across the 128-partition
                        # reduce (~2e-4 on the denominator).
                        esum = sm.tile([128, 512], BF16, tag="esum",
                                       name="esum", bufs=2)
                        if nkt == 4:
                            m0 = sm.tile([128, 512], BF16, tag="m0",
                                         name="m0", bufs=2)
                            m1 = sm.tile([128, 512], BF16, tag="m1",
                                         name="m1", bufs=2)
                            nc.vector.tensor_add(m0[:], ests[0][:],
                                                 ests[1][:])
                            nc.vector.tensor_add(m1[:], ests[2][:],
                                                 ests[3][:])
                            nc.vector.tensor_add(esum[:], m0[:], m1[:])
                        else:
                            chains = []
                            for cc in range(4):
                                idxs = list(range(cc, nkt, 4))
                                t = sm.tile([128, 512], BF16, tag=f"ch{cc}",
                                            name=f"ch{cc}", bufs=2)
                                nc.vector.tensor_add(t[:], ests[idxs[0]][:],
                                                     ests[idxs[1]][:])
                                for kt in idxs[2:]:
                                    nc.vector.tensor_add(t[:], t[:],
                                                         ests[kt][:])
                                chains.append(t)
                            m0 = sm.tile([128, 512], BF16, tag="m0",
                                         name="m0", bufs=2)
                            m1 = sm.tile([128, 512], BF16, tag="m1",
                                         name="m1", bufs=2)
                            nc.vector.tensor_add(m0[:], chains[0][:],
                                                 chains[1][:])
                            nc.vector.tensor_add(m1[:], chains[2][:],
                                                 chains[3][:])
                            nc.vector.tensor_add(esum[:], m0[:], m1[:])
                        # finish the previous block first: its reciprocal
                        # has had a whole block to complete, so the rdb
                        # matmul won't stall PE
                        flush_av()
                        # pct matmuls after the Esum chain so the DVE work
                        # overlaps the PE accumulation
                        pct = pavp.tile([128, 512], F32, tag="pct",
                                        name="pct")
                        for kt in range(nkt):
                            nc.tensor.matmul(
                                pct[:],
                                vt[:, kt, h * DK:(h + 1) * DK],
                                ests[kt][:],
                                start=(kt == 0),
                                stop=(kt == nkt - 1),
                            )
                        # denominator on PE: ones-reduce to [1,512], recip;
                        # the broadcast + scale happen at the next flush
                        denT = dnp.tile([1, 512], F32, tag="dn", name="denT")
                        nc.tensor.matmul(denT[:], onec[:], esum[:],
                                         start=True, stop=True)
                        rdenT = sm.tile([1, 512], BF16, tag="rdT",
                                        name="rdT", bufs=2)
                        with nc.allow_low_precision(
                                reason="bf16 1/den feeds a bf16 matmul; "
                                       "0.1% rms on den is ~25x under gate"):
                            nc.vector.reciprocal(rdenT[:], denT[:])
                        pend_av.append((h, js, pct, rdenT))

                    # ---- phase A: all projections (QK fp8-DR + V),
                    # interleaved so DMA paces under PE work ----
                    w_cur = qk_w_load(0)
                    qk_dr(0, w_cur, inline_x=True)
                    w_cur = qk_w_load(1)
                    qk_dr(1, w_cur)
                    v_pass(0)
                    w_cur = qk_w_load(2)
                    qk_dr(2, w_cur)
                    v_pass(4)
                    w_cur = qk_w_load(3)
                    qk_dr(3, w_cur)
                    v_pass(8)
                    v_pass(12)
                    es1.close()

                    # ---- phase B: attention rounds + output projection ----
                    with (
                        tc.tile_pool(name="est", bufs=4) as estp_,
                        tc.tile_pool(name="sm", bufs=4) as sm_,
                        tc.tile_pool(name="pst", bufs=3,
                                     space="PSUM") as pstp_,
                        tc.tile_pool(name="pav", bufs=2,
                                     space="PSUM") as pavp_,
                        tc.tile_pool(name="dn", bufs=1, space="PSUM") as dnp_,
                        tc.tile_pool(name="pso", bufs=2, space="PSUM") as psop,
                        tc.tile_pool(name="lt", bufs=1) as ltp,
                        tc.tile_pool(name="ot", bufs=4) as otp,
                    ):
                        estp, sm, pstp, pavp, dnp = (estp_, sm_, pstp_,
                                                     pavp_, dnp_)
                        wot = ltp.tile([128, HPC, D], BF16, tag="wot")
                        wo_src = woT_d.ap().rearrange("(c p) e -> p c e",
                                                      p=128)
                        for ct in range(HPC):
                            oq = nc.sync if ct % 2 == 0 else nc.scalar
                            oq.dma_start(wot[:, ct, :], wo_src[:, ct, :])

                        def outproj_qt(qt):
                            # partial out rows qt*128.. for this core's
                            # heads; 4 psum evictions, one store DMA
                            ot = otp.tile([128, D], BF16, tag="ot",
                                          name="ot")
                            for eb in range(NQB):
                                pso = psop.tile([128, 512], F32,
                                                tag="pso", name="pso")
                                for ct in range(HPC):
                                    nc.tensor.matmul(
                                        pso[:],
                                        cxt[ct][:, qt * 128:(qt + 1) * 128],
                                        wot[:, ct, eb * 512:(eb + 1) * 512],
                                        start=(ct == 0),
                                        stop=(ct == HPC - 1),
                                    )
                                nc.vector.tensor_copy(
                                    ot[:, eb * 512:(eb + 1) * 512], pso[:])
                            oq = nc.sync if qt % 2 == 0 else nc.scalar
                            oq.dma_start(
                                out_d.ap()[qt * 128:(qt + 1) * 128, :],
                                ot[:])

                        # round 0: no outproj filler yet, pipeline depth 2
                        e0 = attn_scores(0, 0)
                        e1 = attn_scores(1, 0)
                        attn_av(0, 0, e0)
                        e2 = attn_scores(2, 0)
                        attn_av(1, 0, e1)
                        e3 = attn_scores(3, 0)
                        attn_av(2, 0, e2)
                        attn_av(3, 0, e3)
                        flush_av()
                        # rounds 1-3: depth 3 + outproj chunks as PE filler
                        for J in range(1, NQB):
                            qbase = (J - 1) * 4
                            pend_b = []
                            for h in range(HPC):
                                pend_b.append((h, attn_scores(h, J)))
                                outproj_qt(qbase + h)
                                if len(pend_b) > 2:
                                    hh, ee = pend_b.pop(0)
                                    attn_av(hh, J, ee)
                            for hh, ee in pend_b:
                                attn_av(hh, J, ee)
                            flush_av()
                        for qt in range(12, 16):
                            outproj_qt(qt)

    nc.compile()
    return nc


def get_nc():
    global _NC
    if _NC is None:
        _NC = _build_program()
    return _NC


def make_in_maps(x, wq, wk, wv, wo, token_positions):
    x = np.asarray(x, dtype=np.float32)
    wq = np.asarray(wq, dtype=np.float32)
    wk = np.asarray(wk, dtype=np.float32)
    wv = np.asarray(wv, dtype=np.float32)
    wo = np.asarray(wo, dtype=np.float32)
    pos = np.asarray(token_positions).astype(np.float64)

    bf = ml_dtypes.bfloat16
    f8 = ml_dtypes.float8_e4m3
    perm = np.concatenate([np.arange(0, DK, 2), np.arange(1, DK, 2)])
    WS = 2048.0              # fp8 weight scale (clears e4m3 denormal floor)
    f = DK ** -0.25 / WS     # undone at RoPE eviction via the cos/sin tables

    j = np.arange(DK // 2, dtype=np.float64)
    ang = pos[None, :] / (THETA ** (j[:, None] / (DK // 2)))
    cosv, sinv = np.cos(ang), np.sin(ang)
    A = np.concatenate([cosv, cosv], 0) * f        # [128, S]
    Bs = np.concatenate([sinv, -sinv], 0) * f      # [128, S]
    cs = np.ascontiguousarray(
        np.stack([A, Bs], 1)).astype(bf)           # [128, 2, S]

    kk = np.arange(128)[:, None]
    mm = np.arange(896)[None, :]
    maskbig = (mm >= kk + 384).astype(bf)
    xTb = [np.ascontiguousarray(x[b].T).astype(bf) for b in range(2)]
    x8b = [np.ascontiguousarray(x[b].T).reshape(16, 128, S).astype(f8)
           for b in range(2)]

    in_maps = []
    for core in range(NCORES):
        b, g = core // HPC, core % HPC
        rows = slice(g * C, (g + 1) * C)
        wq_s = wq[rows].reshape(HPC, DK, D)[:, perm].reshape(C, D) * WS
        wk_s = wk[rows].reshape(HPC, DK, D)[:, perm].reshape(C, D) * WS

        def tile_qk8(w_s):
            # [C, D] -> W.T [D, C] -> [h, p, g, j, c] for DoubleRow pairs
            wt = w_s.T.reshape(8, 2, 128, HPC, DK)
            return np.ascontiguousarray(wt.transpose(3, 2, 0, 1, 4)).astype(f8)

        wvt = wv[rows].T.reshape(16, 128, C)
        in_maps.append({
            "xT": xTb[b],
            "x8": x8b[b],
            "wq8": tile_qk8(wq_s),
            "wk8": tile_qk8(wk_s),
            "wvT": np.ascontiguousarray(wvt.transpose(1, 0, 2)).astype(bf),
            "woT": np.ascontiguousarray(wo[:, rows].T).astype(bf),
            "cs": cs,
            "maskbig": maskbig,
        })
    return in_maps


def kernel(x, wq, wk, wv, wo, token_positions):
    nc = get_nc()
    in_maps = make_in_maps(x, wq, wk, wv, wo, token_positions)
    res = bass_utils.run_bass_kernel_spmd(
        nc, in_maps, core_ids=list(range(NCORES)))
    out = np.zeros((2, S, D), dtype=np.float32)
    for core in range(NCORES):
        out[core // HPC] += res.results[core]["out"]
    return out



# revision 72
# speedup vs baseline: 1.3284x; 1.1927x over previous
"""Trainium2 Bass kernel for multi-head self-attention with RoPE.

Problem: x[2,2048,2048] f32, Wq/Wk/Wv/Wo [2048,2048], causal MHA, 16 heads,
dk=128, RoPE on Q/K. Sharding: 8 cores = 2 batches x 4 head-groups
(4 heads/core). Each core computes its batch's partial output projection for
its 4 heads; host sums the 4 partials per batch.

Device-side scheme (per core, all matmuls bf16 with f32 PSUM accumulation):
  - host pre-transposes x -> xT [D,S] and weight slices; RoPE pair
    de-interleave is folded into a row permutation of Wq/Wk so the rotation
    becomes partition-block ops; 1/sqrt(dk) folded into Wq/Wk.
  - QT/KT [dk,S] = W-slice^T-tiles @ xT-tiles (+RoPE, bf16 vector ops)
  - V [S,dk] with a ones column appended (interleaved [.,516] layout)
  - ST tiles [k,q] = KT-slice^T @ QT; exp on ScalarE; causal mask only on
    diagonal tiles (multiply by precomputed triangular mask)
  - ctx [q, dk+1] = expST^T @ V_aug; column dk holds the softmax denominator
  - ctx scaled by 1/denom during PSUM eviction, transposed via PE to ctxT
  - out [S, E] = ctxT^T @ WoT-slice, accumulated over the 4 head-chunks
Phases are emitted interleaved (projection passes between attention blocks)
so projection matmuls fill PE gaps while ScalarE computes exp.
"""
from contextlib import ExitStack

import numpy as np
import ml_dtypes

try:
    import concourse.bass as bass  # noqa: F401
except ImportError:  # fresh grading dir: repo lives at /opt/trn_rl_repo
    import sys
    sys.path.insert(0, "/opt/trn_rl_repo")

import concourse.bass as bass
import concourse.bass_isa as bass_isa
import concourse.mybir as mybir
import concourse.tile as tile
from concourse import bacc, bass_utils

BF16 = mybir.dt.bfloat16
F32 = mybir.dt.float32
FP8 = mybir.dt.float8e4
PM_DR = mybir.MatmulPerfMode.DoubleRow
AF = mybir.ActivationFunctionType

D = 2048          # model dim
S = 2048          # sequence length
DK = 128          # head dim
HPC = 4           # heads per core
C = HPC * DK      # per-core feature slice = 512
THETA = 10000.0
NCORES = 8

_NC = None  # cached compiled Bass module


def _build_program(repeat=1):
    nc = bacc.Bacc("TRN2", debug=False, num_devices=NCORES)

    xT_d = nc.dram_tensor("xT", [D, S], BF16, kind="ExternalInput")
    # fp8 copies for the Q/K projections (DoubleRow): x8[t, p, s] = xT[t*128+p, s];
    # wq8/wk8[h, p, g, j, c] = (WqT_perm * 2048)[(2g+j)*128+p, h*DK+c]
    x8_d = nc.dram_tensor("x8", [D // 128, 128, S], FP8, kind="ExternalInput")
    wq8_d = nc.dram_tensor("wq8", [HPC, 128, 8, 2, DK], FP8,
                           kind="ExternalInput")
    wk8_d = nc.dram_tensor("wk8", [HPC, 128, 8, 2, DK], FP8,
                           kind="ExternalInput")
    wvT_d = nc.dram_tensor("wvT", [128, D // 128, C], BF16,
                           kind="ExternalInput")
    woT_d = nc.dram_tensor("woT", [C, D], BF16, kind="ExternalInput")
    cs_d = nc.dram_tensor("cs", [128, 2, S], BF16, kind="ExternalInput")
    mask_d = nc.dram_tensor("maskbig", [128, 896], BF16, kind="ExternalInput")
    idn_d = nc.dram_tensor("ident", [128, 128], BF16, kind="ExternalInput")
    out_d = nc.dram_tensor("out", [S, D], BF16, kind="ExternalOutput")

    NT = D // 128         # 16 contraction tiles
    NQB = S // 512        # 4 q blocks

    with tile.TileContext(nc) as tc:
        with tc.tile_pool(name="persist", bufs=1) as pp:
            qts = [pp.tile([128, S], BF16, tag=f"qt{h}", name=f"qt{h}")
                   for h in range(HPC)]
            kts = [pp.tile([128, S], BF16, tag=f"kt{h}", name=f"kt{h}")
                   for h in range(HPC)]
            vt = pp.tile([128, NT, HPC * (DK + 1)], BF16, tag="vt")
            cxt = [pp.tile([128, S], BF16, tag=f"cx{h}", name=f"cx{h}")
                   for h in range(HPC)]
            cst = pp.tile([128, 2, S], BF16, tag="cst")
            msk = pp.tile([128, 896], BF16, tag="msk")
            idn = pp.tile([128, 128], BF16, tag="idn")
            zb = pp.tile([128, 1], F32, tag="zb")

            nc.vector.memset(zb[:], 0.0)

            def load_tables():
                nc.sync.dma_start(cst[:], cs_d.ap())
                nc.scalar.dma_start(msk[:], mask_d.ap())
                nc.scalar.dma_start(idn[:], idn_d.ap())

            tbA = cst[:, 0, :]
            tbB = cst[:, 1, :]

            for _rep in range(repeat):
                with ExitStack() as es0:
                    # phase A pools (projections) -> released before phase B
                    # (attention) opens its pools; LIFO stacks
                    es1 = ExitStack()
                    rp = es1.enter_context(tc.tile_pool(name="rp", bufs=3))
                    xw = es1.enter_context(tc.tile_pool(name="xw", bufs=3))
                    x8p = es1.enter_context(tc.tile_pool(name="x8p", bufs=8))
                    ws = es1.enter_context(tc.tile_pool(name="ws", bufs=2))
                    ps1 = es1.enter_context(
                        tc.tile_pool(name="ps1", bufs=1, space="PSUM"))
                    x8t = []     # per g: fp8 x tile [128, 2, 2048]
                    estp = sm = pstp = pavp = None   # phase B, see below

                    def rope_evict(ps, dest, js):
                        # dest = qsb*A + cross(qsb)*B with A=[cos;cos],
                        # B=[+sin;-sin]; cross-half reads pair same-base
                        # operands (walrus same-base rule for 2-SBUF ops).
                        # psum copy on ScalarE (idle in phase A) so it never
                        # queues behind the DVE rope muls
                        qsb = rp.tile([128, 512], BF16, tag="qsb", name="qsb")
                        nc.scalar.copy(qsb[:], ps[:])
                        nc.vector.tensor_mul(dest[:, js], qsb[:], tbA[:, js])
                        tb = rp.tile([128, 512], BF16, tag="tb", name="tb")
                        nc.vector.tensor_mul(tb[0:64, :], qsb[64:128, :],
                                             tbB[64:128, js])
                        nc.vector.tensor_mul(tb[64:128, :], qsb[0:64, :],
                                             tbB[0:64, js])
                        nc.vector.tensor_add(dest[:, js], dest[:, js], tb[:])

                    def load_x8_tile(g):
                        # one fp8 x tile [128, 2, 2048]; g pairs contraction
                        # tiles (2g, 2g+1); full seq width in one DMA
                        xt = x8p.tile([128, 2, S], FP8, tag="x8", name="x8")
                        xq = nc.scalar if g % 2 == 0 else nc.sync
                        xq.dma_start(
                            xt[:],
                            x8_d.ap()[2 * g:2 * g + 2, :, :]
                            .rearrange("j p n -> p j n"))
                        x8t.append(xt)

                    def qk_w_load(h):
                        # prefetch head h's Q/K fp8 weights (one DMA each)
                        wts = []
                        for which, wdram, xq in (("q", wq8_d, nc.sync),
                                                 ("k", wk8_d, nc.scalar)):
                            wt = ws.tile([128, 8, 2, DK], FP8,
                                         tag=f"w8{which}", name=f"w8{which}")
                            xq.dma_start(wt[:], wdram.ap()[h])
                            wts.append(wt)
                        return wts

                    def qk_dr(h, wts, inline_x=False):
                        # fp8 DoubleRow projection of head h: Q then K,
                        # J-blocks in pairs so evictions overlap matmuls;
                        # h=0 paces the x8 loads tile-by-tile with compute
                        for wi, (wt, dest) in enumerate(zip(wts, (qts, kts))):
                            for pair in range(2):
                                psums = [ps1.tile([128, 512], F32,
                                                  tag=f"pp{pair * 2 + i}",
                                                  name=f"pp{pair * 2 + i}")
                                         for i in range(2)]
                                for g in range(8):
                                    if inline_x and wi == 0 and pair == 0:
                                        load_x8_tile(g)
                                    for i in range(2):
                                        J = pair * 2 + i
                                        nc.tensor.matmul(
                                            psums[i][:],
                                            wt[:, g, :, :],
                                            x8t[g][:, :,
                                                   J * 512:(J + 1) * 512],
                                            start=(g == 0), stop=(g == 7),
                                            perf_mode=PM_DR,
                                        )
                                if inline_x and wi == 0 and pair == 0:
                                    load_tables()
                                for i in range(2):
                                    J = pair * 2 + i
                                    rope_evict(psums[i], dest[h],
                                               slice(J * 512, (J + 1) * 512))

                    def v_pass(lo):
                        # project V k-tiles lo..lo+3 (all heads); x streamed
                        # as [128, 512] column stripes, each used once
                        psums = [ps1.tile([128, 512], F32, tag=f"pv{i}",
                                          name=f"pv{i}") for i in range(4)]
                        for tg in range(NT // 4):
                            wt = ws.tile([128, 4, C], BF16, tag="wv",
                                         name="wv")
                            nc.sync.dma_start(
                                wt[:], wvT_d.ap()[:, tg * 4:(tg + 1) * 4, :])
                            # x stripe for 4 contraction tiles in one DMA
                            xv = xw.tile([128, 4, 512], BF16, tag="xv",
                                         name="xv")
                            xq = nc.scalar if tg % 2 == 0 else nc.sync
                            xq.dma_start(
                                xv[:],
                                xT_d.ap()[tg * 512:(tg + 1) * 512,
                                          lo * 128:(lo + 4) * 128]
                                .rearrange("(tt p) n -> p tt n", p=128))
                            for ti in range(4):
                                t = tg * 4 + ti
                                for i in range(4):
                                    nc.tensor.matmul(
                                        psums[i][:],
                                        xv[:, ti, i * 128:(i + 1) * 128],
                                        wt[:, ti, :],
                                        start=(t == 0),
                                        stop=(t == NT - 1),
                                    )
                        for i in range(4):
                            kt = lo + i
                            vk = vt[:, kt, :].rearrange("p (h c) -> p h c",
                                                        c=DK + 1)
                            nc.scalar.copy(
                                vk[:, :, 0:DK],
                                psums[i][:].rearrange("p (h c) -> p h c",
                                                      c=DK))
                            nc.vector.memset(vk[:, :, DK:DK + 1], 1.0)

                    def attn_scores(h, J):
                        # scores + exp for block (h, J); exp runs on ScalarE
                        # while later-emitted PE work proceeds
                        nkt = 4 * J + 4
                        ests = []
                        for kt in range(nkt):
                            # diag tiles: only columns q >= s*128 are used
                            c0 = max(0, (kt - 4 * J)) * 128
                            pst = pstp.tile([128, 512], F32, tag="pst",
                                            name="pst")
                            nc.tensor.matmul(
                                pst[:, c0:512],
                                kts[h][:, kt * 128:(kt + 1) * 128],
                                qts[h][:, J * 512 + c0:(J + 1) * 512],
                                start=True, stop=True,
                            )
                            est = estp.tile([128, 512], BF16, tag=f"e{kt}",
                                            name=f"e{kt}")
                            nc.scalar.activation(est[:, c0:512],
                                                 pst[:, c0:512], AF.Exp,
                                                 bias=zb[:])
                            if kt >= 4 * J:
                                # triangular mask on the 128-wide diag block
                                nc.vector.tensor_mul(
                                    est[:, c0:c0 + 128],
                                    est[:, c0:c0 + 128],
                                    msk[:, 384:512])
                            ests.append(est)
                        return ests

                    def attn_av(h, J, ests):
                        # AV + normalize + transpose into cxt[h]; transposes
                        # deferred one AV group so the DVE rec/scale chain
                        # hides under the next group's matmuls
                        pend = []

                        def flush_t():
                            qg, cxs = pend.pop(0)
                            ptr = pavp.tile([128, 128], BF16, tag="pav",
                                            name="ptr")
                            nc.tensor.transpose(ptr[:], cxs[:], idn[:])
                            nc.any.tensor_copy(
                                cxt[h][:, qg * 128:(qg + 1) * 128], ptr[:])

                        for s4 in range(4):
                            qg = 4 * J + s4
                            pav = pavp.tile([128, DK + 1], F32,
                                            tag="pav", name="pav")
                            for kt in range(qg + 1):
                                nc.tensor.matmul(
                                    pav[:],
                                    ests[kt][:, s4 * 128:(s4 + 1) * 128],
                                    vt[:, kt, h * (DK + 1):(h + 1) * (DK + 1)],
                                    start=(kt == 0),
                                    stop=(kt == qg),
                                )
                            rec = sm.tile([128, 1], F32, tag="rec", name="rec")
                            nc.vector.reciprocal(rec[:], pav[:, DK:DK + 1])
                            cxs = sm.tile([128, DK], BF16, tag="cxs",
                                          name="cxs")
                            nc.vector.tensor_scalar_mul(
                                cxs[:], pav[:, 0:DK], rec[:])
                            pend.append((qg, cxs))
                            if len(pend) > 1:
                                flush_t()
                        while pend:
                            flush_t()

                    # ---- phase A: all projections (QK fp8-DR + V),
                    # interleaved so DMA paces under PE work ----
                    # v_pass(0) leads: it is compute-dense (one 512KB x
                    # stripe per 16 matmuls) so the fp8 x8 tiles stream in
                    # behind it and qk_dr(0) starts with its data resident
                    w_cur = qk_w_load(0)
                    v_pass(0)
                    for g in range(8):
                        load_x8_tile(g)
                    load_tables()
                    qk_dr(0, w_cur)
                    w_cur = qk_w_load(1)
                    v_pass(4)
                    qk_dr(1, w_cur)
                    w_cur = qk_w_load(2)
                    v_pass(8)
                    qk_dr(2, w_cur)
                    w_cur = qk_w_load(3)
                    v_pass(12)
                    qk_dr(3, w_cur)
                    es1.close()

                    # ---- phase B: attention rounds + output projection ----
                    with (
                        tc.tile_pool(name="est", bufs=4) as estp_,
                        tc.tile_pool(name="sm", bufs=4) as sm_,
                        tc.tile_pool(name="pst", bufs=4,
                                     space="PSUM") as pstp_,
                        tc.tile_pool(name="pav", bufs=2,
                                     space="PSUM") as pavp_,
                        tc.tile_pool(name="pso", bufs=2, space="PSUM") as psop,
                        tc.tile_pool(name="lt", bufs=1) as ltp,
                        tc.tile_pool(name="ot", bufs=4) as otp,
                    ):
                        estp, sm, pstp, pavp = (estp_, sm_, pstp_,
                                                pavp_)
                        wot = ltp.tile([128, HPC, D], BF16, tag="wot")
                        wo_src = woT_d.ap().rearrange("(c p) e -> p c e",
                                                      p=128)
                        for ct in range(HPC):
                            oq = nc.sync if ct % 2 == 0 else nc.scalar
                            oq.dma_start(wot[:, ct, :], wo_src[:, ct, :])

                        def outproj_qt(qt):
                            # partial out rows qt*128.. for this core's
                            # heads; 4 psum evictions, one store DMA
                            ot = otp.tile([128, D], BF16, tag="ot",
                                          name="ot")
                            for eb in range(NQB):
                                pso = psop.tile([128, 512], F32,
                                                tag="pso", name="pso")
                                for ct in range(HPC):
                                    nc.tensor.matmul(
                                        pso[:],
                                        cxt[ct][:, qt * 128:(qt + 1) * 128],
                                        wot[:, ct, eb * 512:(eb + 1) * 512],
                                        start=(ct == 0),
                                        stop=(ct == HPC - 1),
                                    )
                                nc.vector.tensor_copy(
                                    ot[:, eb * 512:(eb + 1) * 512], pso[:])
                            oq = nc.sync if qt % 2 == 0 else nc.scalar
                            oq.dma_start(
                                out_d.ap()[qt * 128:(qt + 1) * 128, :],
                                ot[:])

                        # round 0: no outproj filler yet, pipeline depth 2
                        e0 = attn_scores(0, 0)
                        e1 = attn_scores(1, 0)
                        attn_av(0, 0, e0)
                        e2 = attn_scores(2, 0)
                        attn_av(1, 0, e1)
                        e3 = attn_scores(3, 0)
                        attn_av(2, 0, e2)
                        attn_av(3, 0, e3)
                        # rounds 1-3: depth 3 + outproj chunks as PE filler
                        for J in range(1, NQB):
                            qbase = (J - 1) * 4
                            pend_b = []
                            for h in range(HPC):
                                pend_b.append((h, attn_scores(h, J)))
                                outproj_qt(qbase + h)
                                if len(pend_b) > 2:
                                    hh, ee = pend_b.pop(0)
                                    attn_av(hh, J, ee)
                            for hh, ee in pend_b:
                                attn_av(hh, J, ee)
                        for qt in range(12, 16):
                            outproj_qt(qt)

    nc.compile()
    return nc


def get_nc():
    global _NC
    if _NC is None:
        _NC = _build_program()
    return _NC


def make_in_maps(x, wq, wk, wv, wo, token_positions):
    x = np.asarray(x, dtype=np.float32)
    wq = np.asarray(wq, dtype=np.float32)
    wk = np.asarray(wk, dtype=np.float32)
    wv = np.asarray(wv, dtype=np.float32)
    wo = np.asarray(wo, dtype=np.float32)
    pos = np.asarray(token_positions).astype(np.float64)

    bf = ml_dtypes.bfloat16
    f8 = ml_dtypes.float8_e4m3
    perm = np.concatenate([np.arange(0, DK, 2), np.arange(1, DK, 2)])
    WS = 2048.0              # fp8 weight scale (clears e4m3 denormal floor)
    f = DK ** -0.25 / WS     # undone at RoPE eviction via the cos/sin tables

    j = np.arange(DK // 2, dtype=np.float64)
    ang = pos[None, :] / (THETA ** (j[:, None] / (DK // 2)))
    cosv, sinv = np.cos(ang), np.sin(ang)
    A = np.concatenate([cosv, cosv], 0) * f        # [128, S]
    Bs = np.concatenate([sinv, -sinv], 0) * f      # [128, S]
    cs = np.ascontiguousarray(
        np.stack([A, Bs], 1)).astype(bf)           # [128, 2, S]

    kk = np.arange(128)[:, None]
    mm = np.arange(896)[None, :]
    maskbig = (mm >= kk + 384).astype(bf)
    ident = np.eye(128, dtype=np.float32).astype(bf)
    xTb = [np.ascontiguousarray(x[b].T).astype(bf) for b in range(2)]
    x8b = [np.ascontiguousarray(x[b].T).reshape(16, 128, S).astype(f8)
           for b in range(2)]

    in_maps = []
    for core in range(NCORES):
        b, g = core // HPC, core % HPC
        rows = slice(g * C, (g + 1) * C)
        wq_s = wq[rows].reshape(HPC, DK, D)[:, perm].reshape(C, D) * WS
        wk_s = wk[rows].reshape(HPC, DK, D)[:, perm].reshape(C, D) * WS

        def tile_qk8(w_s):
            # [C, D] -> W.T [D, C] -> [h, p, g, j, c] for DoubleRow pairs
            wt = w_s.T.reshape(8, 2, 128, HPC, DK)
            return np.ascontiguousarray(wt.transpose(3, 2, 0, 1, 4)).astype(f8)

        wvt = wv[rows].T.reshape(16, 128, C)
        in_maps.append({
            "xT": xTb[b],
            "x8": x8b[b],
            "wq8": tile_qk8(wq_s),
            "wk8": tile_qk8(wk_s),
            "wvT": np.ascontiguousarray(wvt.transpose(1, 0, 2)).astype(bf),
            "woT": np.ascontiguousarray(wo[:, rows].T).astype(bf),
            "cs": cs,
            "maskbig": maskbig,
            "ident": ident,
        })
    return in_maps


def kernel(x, wq, wk, wv, wo, token_positions):
    nc = get_nc()
    in_maps = make_in_maps(x, wq, wk, wv, wo, token_positions)
    res = bass_utils.run_bass_kernel_spmd(
        nc, in_maps, core_ids=list(range(NCORES)))
    out = np.zeros((2, S, D), dtype=np.float32)
    for core in range(NCORES):
        out[core // HPC] += res.results[core]["out"]
    return out

